# revision 16
# baseline (speedup 1.0000x reference)
"""Causal self-attention Trainium2 kernel (8 NeuronCores), v3.

Sharding (Megatron-style): core c -> batch b = c//2, head-group g = c%2
(8 of 16 heads). W_q/W_k/W_v column-sliced per head group; W_o row-sliced;
host sums the two partial outputs per batch and adds b_o.

v3 changes over v2 (270751 ns):
  * LAG-2 software pipeline in the attention inner loop: at step g the
    emission order is scores(g+1) -> exp(g) -> ctx(g-2) -> filler, so the
    in-order PE never sits directly behind the exp it just triggered
    (two full steps of slack absorb Act latency + the DVE mask hop).
  * Engine rebalance: normalize muls and out-proj PSUM->SBUF copies move
    to the idle Pool/GPSIMD engine; DVE keeps q/k/v copies, masks, recips.
  * Chunk-0 ramp: attention on head-pair 0 starts right after the u=0
    q/k projection group; u=1/u=2 projections and v tiles feed in as
    paced filler instead of a serial prologue.
  * expp bufs 4->6 to cover the deeper ex-tile liveness.

Per-core kernel layout:
  xt8   [512, 4096]  x^T fp8 pairs: row pair*128+p, col sub*2048+n
  wqk8  [512, 3072]  [Wq'|Wk'] fp8 pairs, x32, strip-permuted columns
  xtb   [1024, 2048] x^T bf16 (v projection)
  wv    [1024, 512]  W_v slice bf16
  wo    [512, 1024]  W_o rows bf16
  out   [2048, 1024] fp16 partial

`reps` repeats the body inside one NEFF for (T(3)-T(1))/2 timing; graded
path is reps=1.
"""

import sys

import numpy as np

sys.path.insert(0, "/opt/trn_rl_repo")

import ml_dtypes

BF16 = ml_dtypes.bfloat16
F8E4 = ml_dtypes.float8_e4m3

D_EMB = 1024
N_SEQ = 2048
HD = 64
NPAIR = 4  # d_emb DoubleRow pairs (256 each)
KT = 8  # d_emb 128-tiles (v projection)
NT = N_SEQ // 128  # 16 n-tiles
QC = N_SEQ // 512  # 4 query chunks
WS = 32.0  # host prescale on W_q/W_k before fp8 quantization
SCALE_EXP = 0.125 / (WS * WS)  # 1/sqrt(64) / (32*32)
# 3-strip head layout: tile u hosts heads HEADS3[u] at partition strips 0/32/64
HEADS3 = ((0, 1, 2), (3, 4, 5), (6, 7))
LAG = 3  # ctx emission lag (steps) behind exp

_CACHE = {}


def _emit_body(nc, tc, mybir, sfx, rep, pools, tiles, xt8_d, wqk8_d, xtb_d, wv_d, wo_d, tri_d, out_d):
    f32 = mybir.dt.float32
    f16 = mybir.dt.float16
    bf16 = mybir.dt.bfloat16
    DR = mybir.MatmulPerfMode.DoubleRow

    expp, rpool, outp, psq_pool, pssc_pool, psctx_pool = pools
    xt8_sb = tiles["xt8"]
    wqk8_sb = tiles["wqk8"]
    xtb_sb = tiles["xtb"]
    wv_sb = tiles["wv"]
    wo_sb = tiles["wo"]
    qt_sb = tiles["qt"]
    kt_sb = tiles["kt"]
    v_sb = tiles["v"]
    ctxt_sb = tiles["ctxt"]
    tri_sb = tiles["tri"]

    def pair3(tile, n_inner):  # [128, 2*n] -> [128, 2, n]
        return tile.rearrange("p (s n) -> p s n", s=2)

    # ---- input DMA, split across queues ----
    # sync/HWDGE queue: tri mask, interleaved xt8/wqk8 pairs (q/k proj
    # ramps with the stream), then wv and wo; gpsimd/SWDGE queue carries
    # xtb in parallel so the v tiles land just in time for the
    # demand-driven v projections (Pool must be free of setup work or the
    # SWDGE descriptor generation delays xtb by ~10us).
    nc.sync.dma_start(out=tri_sb[:], in_=tri_d[:, :])
    for i in range(NPAIR):
        nc.sync.dma_start(out=xt8_sb[i][:], in_=xt8_d[i * 128 : (i + 1) * 128, :])
        nc.sync.dma_start(out=wqk8_sb[i][:], in_=wqk8_d[i * 128 : (i + 1) * 128, :])
    for k in range(KT):
        nc.sync.dma_start(out=wv_sb[k][:], in_=wv_d[k * 128 : (k + 1) * 128, :])
    for k in range(KT):
        nc.sync.dma_start(out=xtb_sb[k][:], in_=xtb_d[k * 128 : (k + 1) * 128, :])
    for p in range(4):
        nc.sync.dma_start(out=wo_sb[p][:], in_=wo_d[p * 128 : (p + 1) * 128, :])

    # ---- work-stream closures ----
    def qk_proj_group3(qc, which, u, s):
        # fp8 strip layout: PSUM = [3 heads x 32 hd-dims | pad]; copy to
        # qt/kt tile u, half s, partitions 0:32*len(HEADS3[u])
        def emit():
            nsl = slice(qc * 512, (qc + 1) * 512)
            ps = psq_pool.tile(
                [128, 512], f32, name=f"pq3{qc}{which}{u}{s}{sfx}", tag="psq"
            )
            base = which * 768 + u * 256 + s * 128
            for pair in range(NPAIR):
                nc.tensor.matmul(
                    ps[:],
                    lhsT=pair3(wqk8_sb[pair], 1536)[:, :, base : base + 128],
                    rhs=pair3(xt8_sb[pair], N_SEQ)[:, :, nsl],
                    start=(pair == 0),
                    stop=(pair == NPAIR - 1),
                    perf_mode=DR,
                )
            np_ = 32 * len(HEADS3[u])
            dst = qt_sb[u] if which == 0 else kt_sb[u]
            nc.vector.tensor_copy(pair3(dst, N_SEQ)[0:np_, s, nsl], ps[0:np_, :])

        return emit

    # v projection, split into 2 emission halves so the PE cost spreads
    # across attention steps; state[nt] holds the psum tile between halves.
    v_ps = {}

    def v_proj_half(nt, half):
        def emit():
            if half == 0:
                v_ps[nt] = psq_pool.tile(
                    [128, 512], f32, name=f"pv{nt}{sfx}", tag="psq"
                )
            ps = v_ps[nt]
            for k in range(4 * half, 4 * half + 4):
                nc.tensor.matmul(
                    ps[:],
                    lhsT=xtb_sb[k][:, nt * 128 : (nt + 1) * 128],
                    rhs=wv_sb[k][:],
                    start=(k == 0),
                    stop=(k == KT - 1),
                )
            if half == 1:
                del v_ps[nt]
                v_view = v_sb[nt].rearrange("p (h c) -> p h c", h=8)
                nc.vector.tensor_copy(
                    v_view[:, :, 0:64], ps.rearrange("p (h c) -> p h c", h=8)
                )

        return emit

    osb_tiles = {}

    def outproj_half(nt, dh):
        def emit():
            if dh == 0:
                osb_tiles[nt] = outp.tile(
                    [128, D_EMB], f16, name=f"osb{nt}{sfx}", tag="osb"
                )
            osb = osb_tiles[nt]
            pso = psq_pool.tile([128, 512], f32, name=f"po{nt}{dh}{sfx}", tag="psq")
            for hp in range(4):
                nc.tensor.matmul(
                    pso[:],
                    lhsT=ctxt_sb[hp][:, nt * 128 : (nt + 1) * 128],
                    rhs=wo_sb[hp][:, dh * 512 : (dh + 1) * 512],
                    start=(hp == 0),
                    stop=(hp == 3),
                )
            nc.vector.tensor_copy(osb[:, dh * 512 : (dh + 1) * 512], pso[:])
            if dh == 1:
                del osb_tiles[nt]
                nc.sync.dma_start(
                    out=out_d[nt * 128 : (nt + 1) * 128, :], in_=osb[:]
                )

        return emit

    # ---- attention ----
    def emit_scores(qc, hp, ki, ps_tiles):
        # ps [128, 1024]: k-tile ki x (2 heads x 512 q); fp8 DoubleRow
        jj = ki - 4 * qc
        t0 = max(0, 128 * jj)
        q0 = qc * 512
        ps = pssc_pool.tile([128, 1024], f32, name=f"sc{qc}_{hp}_{ki}{sfx}", tag="sc")
        ps_tiles[(hp, ki)] = ps
        for h2 in range(2):
            h = 2 * hp + h2
            u, st = h // 3, 32 * (h % 3)
            kt_ap = pair3(kt_sb[u], N_SEQ)
            qt_ap = pair3(qt_sb[u], N_SEQ)
            nc.tensor.matmul(
                ps[:, h2 * 512 + t0 : (h2 + 1) * 512],
                lhsT=kt_ap[st : st + 32, :, ki * 128 : (ki + 1) * 128],
                rhs=qt_ap[st : st + 32, :, q0 + t0 : q0 + 512],
                start=True,
                stop=True,
                perf_mode=DR,
            )

    def emit_exp(qc, hp, ki, ps_tiles, ex_tiles):
        jj = ki - 4 * qc
        t0 = max(0, 128 * jj)
        ps = ps_tiles.pop((hp, ki))
        ex = expp.tile([128, 1024], bf16, name=f"ex{qc}_{hp}_{ki}{sfx}", tag="ex")
        ex_tiles[(hp, ki)] = ex
        if t0:
            nc.scalar.activation(
                ex.rearrange("p (h n) -> p h n", h=2)[:, :, t0:512],
                ps.rearrange("p (h n) -> p h n", h=2)[:, :, t0:512],
                mybir.ActivationFunctionType.Exp,
                scale=float(SCALE_EXP),
            )
        else:
            nc.scalar.activation(
                ex[:], ps[:], mybir.ActivationFunctionType.Exp,
                scale=float(SCALE_EXP),
            )
        if jj >= 0:  # triangular mask on the diagonal 128-block (Pool:
            # SBUF-only op, keeps DVE free; LAG hides the extra hop)
            for h2 in range(2):
                blk = slice(h2 * 512 + t0, h2 * 512 + t0 + 128)
                nc.gpsimd.tensor_mul(ex[:, blk], ex[:, blk], tri_sb[:])

    def emit_ctx(qc, hp, ki, nk, ex_tiles, ctx_ps):
        jj = ki - 4 * qc
        t0 = max(0, 128 * jj)
        q0 = qc * 512
        if ki == 0:
            for h2 in range(2):
                ctx_ps[(hp, h2)] = psctx_pool.tile(
                    [128, 512], f32, name=f"ctx{qc}_{hp}_{h2}{sfx}", tag="ctx"
                )
        ex = ex_tiles.pop((hp, ki))
        for h2 in range(2):
            h = 2 * hp + h2
            nc.tensor.matmul(
                ctx_ps[(hp, h2)][:, t0:512],
                lhsT=v_sb[ki][:, h * 128 : (h + 1) * 128],
                rhs=ex[:, h2 * 512 + t0 : (h2 + 1) * 512],
                start=(ki == 0),
                stop=(ki == nk - 1),
            )

    def emit_normalize(qc, hp, ctx_ps):
        q0 = qc * 512
        for h2 in range(2):
            cp = ctx_ps.pop((hp, h2))
            rec = rpool.tile([64, 512], f32, name=f"rec{qc}_{hp}_{h2}{sfx}", tag="rec")
            nc.vector.reciprocal(rec[:], cp[64:128, :])
            nc.vector.tensor_mul(
                ctxt_sb[hp][h2 * 64 : h2 * 64 + 64, q0 : q0 + 512],
                cp[0:64, :],
                rec[:],
            )

    # ---- global rolling pipeline ----
    # One step per (qc, hp, ki) group, across all chunks: at step s emit
    # scores(s+1) -> exp(s) -> lagged ctx(s-LAG) (+normalize at stream
    # ends, rolling across stream/phase boundaries) -> filler pulled from
    # a deadline-sorted queue under a per-step PE-headroom credit, so PE
    # never outruns the Act exp pace for long.
    order = [0, 1, 2, 3]
    G = []
    phase_start = {}
    for qc in order:
        nk = 4 * qc + 4
        phase_start[qc] = len(G)
        for hp in range(4):
            for ki in range(nk):
                G.append((qc, hp, ki, nk))
    S = len(G)

    # filler queue entries: [deadline, seq, cost_ns, emit_fn, carry_fn]
    queue = []
    seqn = [0]

    def push(deadline, cost, emit, carry=None, carry_cost=0.0):
        queue.append([deadline, seqn[0], cost, emit, carry, carry_cost])
        seqn[0] += 1
        queue.sort()

    # q/k projection groups: u=0 needed at phase start, u=1 by stream 1,
    # u=2 by stream 3 (head->tile mapping per HEADS3).
    for qc in order:
        nk = 4 * qc + 4
        for u, margin in ((0, -4), (1, nk - 4), (2, 3 * nk - 4)):
            if qc == order[0] and u == 0:
                continue  # prologue below
            for w in (0, 1):
                for s in (0, 1):
                    push(
                        phase_start[qc] + margin,
                        428.0,
                        qk_proj_group3(qc, w, u, s),
                    )
    # v tiles: v[k] first consumed by ctx of (qc=k//4, hp=0, ki=k), which
    # pops at phase_start + k + LAG; both halves + copy need a step margin.
    for k in range(NT):
        push(
            max(phase_start[k // 4] + k + LAG - 1, 8),
            853.0,
            v_proj_half(k, 0),
            carry=v_proj_half(k, 1),
            carry_cost=853.0,
        )

    def headroom(w):
        # Act exp pace minus this step's scores+ctx PE cost (ns)
        return 0.4167 * w + 185.0

    # prologue: u=0 q/k groups for the first chunk
    for w in (0, 1):
        for s in (0, 1):
            qk_proj_group3(order[0], w, 0, s)()

    ps_tiles = {}
    ex_tiles = {}
    ctx_ps = {}
    pending = []  # (qc, hp, ki, nk) with exp emitted, ctx not yet emitted
    credit = 0.0
    carry = None
    carry_cost = 0.0

    def pop_ctx():
        cqc, chp, cki, cnk = pending.pop(0)
        emit_ctx(cqc, chp, cki, cnk, ex_tiles, ctx_ps)
        if cki == cnk - 1:
            emit_normalize(cqc, chp, ctx_ps)
            if chp == 3:  # chunk fully normalized: queue its out-proj
                for nt in range(4 * cqc, 4 * cqc + 4):
                    for dh in range(2):
                        push(10 ** 9, 960.0, outproj_half(nt, dh))

    qc0, hp0, ki0, nk0 = G[0]
    emit_scores(qc0, hp0, ki0, ps_tiles)
    for s in range(S):
        qc, hp, ki, nk = G[s]
        if s + 1 < S:
            nqc, nhp, nki, _ = G[s + 1]
            emit_scores(nqc, nhp, nki, ps_tiles)
        emit_exp(qc, hp, ki, ps_tiles, ex_tiles)
        pending.append(G[s])

        credit += headroom(512 - max(0, 128 * (ki - 4 * qc)))
        if carry is not None:  # second half of a split v projection
            carry()
            credit -= carry_cost
            carry = None

        # during the DMA-bound ramp, let pending grow (deep LAG) so PE's
        # wait queue never jams on v tiles; catch up 2 pops/step after
        eff_lag = 10 if s < 16 else LAG
        npop = 0
        while len(pending) > eff_lag and npop < 2:
            pop_ctx()
            npop += 1

        # forced (deadline) and voluntary (credit) filler pulls; a v pull
        # sets `carry` and ends the step so its psum chain stays tight.
        pulled = 0.0
        while queue and carry is None:
            deadline, _, cost, emit, c_fn, c_cost = queue[0]
            if deadline > s and (credit < cost or pulled >= 1100.0):
                break
            queue.pop(0)
            emit()
            credit -= cost
            pulled += cost
            if c_fn is not None:
                carry = c_fn
                carry_cost = c_cost

    # drain: trailing lagged ctx + normalizes, then remaining fillers
    while pending:
        pop_ctx()
    if carry is not None:
        carry()
        carry = None
    while queue:
        _, _, _, emit, c_fn, _ = queue.pop(0)
        emit()
        if c_fn is not None:
            c_fn()


def _build_module(reps=1):
    import concourse.bacc as bacc
    import concourse.mybir as mybir
    import concourse.tile as tile

    f16 = mybir.dt.float16
    f32 = mybir.dt.float32
    bf16 = mybir.dt.bfloat16
    f8 = mybir.dt.float8e4

    nc = bacc.Bacc()
    xt8_d = nc.dram_tensor("xt8", [512, 2 * N_SEQ], f8, kind="ExternalInput")
    wqk8_d = nc.dram_tensor("wqk8", [512, 3072], f8, kind="ExternalInput")
    xtb_d = nc.dram_tensor("xtb", [D_EMB, N_SEQ], bf16, kind="ExternalInput")
    wv_d = nc.dram_tensor("wv", [D_EMB, 512], bf16, kind="ExternalInput")
    wo_d = nc.dram_tensor("wo", [512, D_EMB], bf16, kind="ExternalInput")
    tri_d = nc.dram_tensor("tri", [128, 128], bf16, kind="ExternalInput")
    out_d = nc.dram_tensor("out", [N_SEQ, D_EMB], f16, kind="ExternalOutput")

    with tile.TileContext(nc) as tc:
        with (
            tc.tile_pool(name="persist", bufs=1) as persist,
            tc.tile_pool(name="expp", bufs=12) as expp,
            tc.tile_pool(name="rpool", bufs=4) as rpool,
            tc.tile_pool(name="outp", bufs=3) as outp,
            tc.tile_pool(name="psq", bufs=2, space="PSUM") as psq_pool,
            tc.tile_pool(name="pssc", bufs=2, space="PSUM") as pssc_pool,
            tc.tile_pool(name="psctx", bufs=2, space="PSUM") as psctx_pool,
        ):
            pools = (expp, rpool, outp, psq_pool, pssc_pool, psctx_pool)
            tiles = {
                "xt8": [persist.tile([128, 2 * N_SEQ], f8, name=f"xt8_{i}") for i in range(NPAIR)],
                "wqk8": [
                    persist.tile([128, 3072], f8, name=f"wqk8_{i}")
                    for i in range(NPAIR)
                ],
                "xtb": [persist.tile([128, N_SEQ], bf16, name=f"xtb{k}") for k in range(KT)],
                "wv": [persist.tile([128, 512], bf16, name=f"wv{k}") for k in range(KT)],
                "wo": [persist.tile([128, D_EMB], bf16, name=f"wo{p}") for p in range(4)],
                "qt": [persist.tile([128, 2 * N_SEQ], f8, name=f"qt{u}") for u in range(3)],
                "kt": [persist.tile([128, 2 * N_SEQ], f8, name=f"kt{u}") for u in range(3)],
                # v per n-tile [128, 1024]: head h -> cols h*128:h*128+64 = v_h,
                # cols h*128+64:h*128+128 = 1.0 (softmax denominator ones-trick)
                "v": [persist.tile([128, 1024], bf16, name=f"v{nt}") for nt in range(NT)],
                "ctxt": [persist.tile([128, N_SEQ], bf16, name=f"ctxt{p}") for p in range(4)],
                "tri": persist.tile([128, 128], bf16, name="tri"),
            }
            # ones blocks, once for all reps — on DVE so the Pool engine
            # is free to generate the xtb SWDGE descriptors immediately
            for nt in range(NT):
                ones_view = tiles["v"][nt].rearrange("p (h c) -> p h c", h=8)
                nc.gpsimd.memset(ones_view[:, :, 64:128], 1.0)
            for rep in range(reps):
                _emit_body(
                    nc, tc, mybir, f"_r{rep}" if reps > 1 else "", rep, pools, tiles,
                    xt8_d, wqk8_d, xtb_d, wv_d, wo_d, tri_d, out_d,
                )

    if not nc.is_finalized():
        nc.finalize()
    return nc


def _get_module(reps=1):
    key = f"nc{reps}"
    if key not in _CACHE:
        _CACHE[key] = _build_module(reps)
    return _CACHE[key]


def _pairs(a, ncols):
    # [1024, ncols] -> [512, 2*ncols]: row pair*128+p, col sub*ncols+c
    return (
        a.reshape(NPAIR, 2, 128, ncols).transpose(0, 2, 1, 3).reshape(512, 2 * ncols)
    )


def _f8(a):
    return np.clip(a, -240.0, 240.0).astype(F8E4)


def make_in_maps(x, W_q, W_k, W_v, W_o):
    x = np.asarray(x, np.float32)
    in_maps = []
    for c in range(8):
        b, g = c // 2, c % 2
        gs = slice(g * 512, (g + 1) * 512)
        xT = np.ascontiguousarray(x[b].T)  # [1024, 2048]
        wq = np.asarray(W_q[:, gs], np.float32) * WS
        wk = np.asarray(W_k[:, gs], np.float32) * WS

        # strip layout: per tile u, half s: [head, 32 dims] + pad to 128
        def strip_cols(w):
            cols = []
            for u in range(3):
                for s in (0, 1):
                    blk = np.concatenate(
                        [w[:, 64 * h + 32 * s : 64 * h + 32 * s + 32] for h in HEADS3[u]],
                        axis=1,
                    )
                    if blk.shape[1] < 128:
                        blk = np.concatenate(
                            [blk, np.zeros((1024, 128 - blk.shape[1]), np.float32)],
                            axis=1,
                        )
                    cols.append(blk)
            return np.concatenate(cols, axis=1)  # [1024, 768]

        wqk = np.concatenate([strip_cols(wq), strip_cols(wk)], axis=1)  # [1024, 1536]
        # tri[k_local, q_local] = 1.0 if q_local >= k_local else 0
        tri = np.tril(np.ones((128, 128), np.float32)).T
        in_maps.append(
            {
                "xt8": _f8(_pairs(xT, N_SEQ)),
                "wqk8": _f8(_pairs(wqk, wqk.shape[1])),
                "xtb": xT.astype(BF16),
                "wv": np.ascontiguousarray(W_v[:, gs]).astype(BF16),
                "wo": np.ascontiguousarray(W_o[gs, :]).astype(BF16),
                "tri": np.ascontiguousarray(tri).astype(BF16),
            }
        )
    return in_maps


def kernel(x, W_q, W_k, W_v, W_o, b_o):
    from concourse.bass_utils import run_bass_kernel_spmd

    nc = _get_module()
    in_maps = make_in_maps(x, W_q, W_k, W_v, W_o)
    res = run_bass_kernel_spmd(nc, in_maps, core_ids=list(range(8)))

    out = np.empty((4, N_SEQ, D_EMB), np.float32)
    for b in range(4):
        out[b] = (
            res.results[2 * b]["out"].astype(np.float32)
            + res.results[2 * b + 1]["out"].astype(np.float32)
            + np.asarray(b_o, np.float32)[None, :]
        )
    return out


# revision 20
# speedup vs baseline: 1.5168x; 1.5168x over previous
"""Causal self-attention Trainium2 kernel (8 NeuronCores), v3.

Sharding (Megatron-style): core c -> batch b = c//2, head-group g = c%2
(8 of 16 heads). W_q/W_k/W_v column-sliced per head group; W_o row-sliced;
host sums the two partial outputs per batch and adds b_o.

v3 changes over v2 (270751 ns):
  * LAG-2 software pipeline in the attention inner loop: at step g the
    emission order is scores(g+1) -> exp(g) -> ctx(g-2) -> filler, so the
    in-order PE never sits directly behind the exp it just triggered
    (two full steps of slack absorb Act latency + the DVE mask hop).
  * Engine rebalance: normalize muls and out-proj PSUM->SBUF copies move
    to the idle Pool/GPSIMD engine; DVE keeps q/k/v copies, masks, recips.
  * Chunk-0 ramp: attention on head-pair 0 starts right after the u=0
    q/k projection group; u=1/u=2 projections and v tiles feed in as
    paced filler instead of a serial prologue.
  * expp bufs 4->6 to cover the deeper ex-tile liveness.

Per-core kernel layout:
  xt8   [512, 4096]  x^T fp8 pairs: row pair*128+p, col sub*2048+n
  wqk8  [512, 3072]  [Wq'|Wk'] fp8 pairs, x32, strip-permuted columns
  xtb   [1024, 2048] x^T bf16 (v projection)
  wv    [1024, 512]  W_v slice bf16
  wo    [512, 1024]  W_o rows bf16
  out   [2048, 1024] fp16 partial

`reps` repeats the body inside one NEFF for (T(3)-T(1))/2 timing; graded
path is reps=1.
"""

import sys

import numpy as np

sys.path.insert(0, "/opt/trn_rl_repo")

import ml_dtypes

BF16 = ml_dtypes.bfloat16
F8E4 = ml_dtypes.float8_e4m3

D_EMB = 1024
N_SEQ = 2048
HD = 64
NPAIR = 4  # d_emb DoubleRow pairs (256 each)
KT = 8  # d_emb 128-tiles (v projection)
NT = N_SEQ // 128  # 16 n-tiles
QC = N_SEQ // 512  # 4 query chunks
WS = 32.0  # host prescale on W_q/W_k before fp8 quantization
SCALE_EXP = 0.125 / (WS * WS)  # 1/sqrt(64) / (32*32)
# 3-strip head layout: tile u hosts heads HEADS3[u] at partition strips 0/32/64
HEADS3 = ((0, 1, 2), (3, 4, 5), (6, 7))
LAG = 3  # ctx emission lag (steps) behind exp

_CACHE = {}


def _emit_body(nc, tc, mybir, sfx, rep, pools, tiles, xt8_d, wqk8_d, xtb_d, wv_d, wo_d, tri_d, out_d):
    f32 = mybir.dt.float32
    f16 = mybir.dt.float16
    bf16 = mybir.dt.bfloat16
    DR = mybir.MatmulPerfMode.DoubleRow

    expp, rpool, outp, psq_pool, pssc_pool, psctx_pool = pools
    xt8_sb = tiles["xt8"]
    wqk8_sb = tiles["wqk8"]
    xtb_sb = tiles["xtb"]
    wv_sb = tiles["wv"]
    wo_sb = tiles["wo"]
    qt_sb = tiles["qt"]
    kt_sb = tiles["kt"]
    v_sb = tiles["v"]
    ctxt_sb = tiles["ctxt"]
    tri_sb = tiles["tri"]

    def pair3(tile, n_inner):  # [128, 2*n] -> [128, 2, n]
        return tile.rearrange("p (s n) -> p s n", s=2)

    # ---- input DMA, split across queues ----
    # sync/HWDGE queue: tri mask, interleaved xt8/wqk8 pairs (q/k proj
    # ramps with the stream), then wv and wo; gpsimd/SWDGE queue carries
    # xtb in parallel so the v tiles land just in time for the
    # demand-driven v projections (Pool must be free of setup work or the
    # SWDGE descriptor generation delays xtb by ~10us).
    nc.sync.dma_start(out=tri_sb[:], in_=tri_d[:, :])
    for i in range(NPAIR):
        nc.sync.dma_start(out=xt8_sb[i][:], in_=xt8_d[i * 128 : (i + 1) * 128, :])
        nc.sync.dma_start(out=wqk8_sb[i][:], in_=wqk8_d[i * 128 : (i + 1) * 128, :])
    for k in range(KT):
        nc.sync.dma_start(out=wv_sb[k][:], in_=wv_d[k * 128 : (k + 1) * 128, :])
    for k in range(KT):
        nc.sync.dma_start(out=xtb_sb[k][:], in_=xtb_d[k * 128 : (k + 1) * 128, :])
    for p in range(4):
        nc.sync.dma_start(out=wo_sb[p][:], in_=wo_d[p * 128 : (p + 1) * 128, :])

    # ---- work-stream closures ----
    def qk_proj_group3(qc, which, u, s):
        # fp8 strip layout: PSUM = [3 heads x 32 hd-dims | pad]; copy to
        # qt/kt tile u, half s, partitions 0:32*len(HEADS3[u])
        def emit():
            nsl = slice(qc * 512, (qc + 1) * 512)
            ps = psq_pool.tile(
                [128, 512], f32, name=f"pq3{qc}{which}{u}{s}{sfx}", tag="psq"
            )
            base = which * 768 + u * 256 + s * 128
            for pair in range(NPAIR):
                nc.tensor.matmul(
                    ps[:],
                    lhsT=pair3(wqk8_sb[pair], 1536)[:, :, base : base + 128],
                    rhs=pair3(xt8_sb[pair], N_SEQ)[:, :, nsl],
                    start=(pair == 0),
                    stop=(pair == NPAIR - 1),
                    perf_mode=DR,
                )
            np_ = 32 * len(HEADS3[u])
            dst = qt_sb[u] if which == 0 else kt_sb[u]
            nc.vector.tensor_copy(pair3(dst, N_SEQ)[0:np_, s, nsl], ps[0:np_, :])

        return emit

    # v projection, split into 2 emission halves so the PE cost spreads
    # across attention steps; state[nt] holds the psum tile between halves.
    v_ps = {}

    def v_proj_half(nt, half):
        def emit():
            if half == 0:
                v_ps[nt] = psq_pool.tile(
                    [128, 512], f32, name=f"pv{nt}{sfx}", tag="psq"
                )
            ps = v_ps[nt]
            for k in range(4 * half, 4 * half + 4):
                nc.tensor.matmul(
                    ps[:],
                    lhsT=xtb_sb[k][:, nt * 128 : (nt + 1) * 128],
                    rhs=wv_sb[k][:],
                    start=(k == 0),
                    stop=(k == KT - 1),
                )
            if half == 1:
                del v_ps[nt]
                v_view = v_sb[nt].rearrange("p (h c) -> p h c", h=8)
                nc.vector.tensor_copy(
                    v_view[:, :, 0:64], ps.rearrange("p (h c) -> p h c", h=8)
                )

        return emit

    osb_tiles = {}
    drain_mode = []  # non-empty once the epilogue drain starts

    def outproj_half(nt, dh):
        def emit():
            if dh == 0:
                osb_tiles[nt] = outp.tile(
                    [128, D_EMB], f16, name=f"osb{nt}{sfx}", tag="osb"
                )
            osb = osb_tiles[nt]
            pso = psq_pool.tile([128, 512], f32, name=f"po{nt}{dh}{sfx}", tag="psq")
            for hp in range(4):
                nc.tensor.matmul(
                    pso[:],
                    lhsT=ctxt_sb[hp][:, nt * 128 : (nt + 1) * 128],
                    rhs=wo_sb[hp][:, dh * 512 : (dh + 1) * 512],
                    start=(hp == 0),
                    stop=(hp == 3),
                )
            nc.vector.tensor_copy(osb[:, dh * 512 : (dh + 1) * 512], pso[:])
            if dh == 1:
                del osb_tiles[nt]
                nc.sync.dma_start(
                    out=out_d[nt * 128 : (nt + 1) * 128, :], in_=osb[:]
                )

        return emit

    # ---- attention ----
    def emit_scores(qc, hp, ki, ps_tiles):
        # ps [128, 1024]: k-tile ki x (2 heads x 512 q); fp8 DoubleRow
        jj = ki - 4 * qc
        t0 = max(0, 128 * jj)
        q0 = qc * 512
        ps = pssc_pool.tile([128, 1024], f32, name=f"sc{qc}_{hp}_{ki}{sfx}", tag="sc")
        ps_tiles[(hp, ki)] = ps
        for h2 in range(2):
            h = 2 * hp + h2
            u, st = h // 3, 32 * (h % 3)
            kt_ap = pair3(kt_sb[u], N_SEQ)
            qt_ap = pair3(qt_sb[u], N_SEQ)
            nc.tensor.matmul(
                ps[:, h2 * 512 + t0 : (h2 + 1) * 512],
                lhsT=kt_ap[st : st + 32, :, ki * 128 : (ki + 1) * 128],
                rhs=qt_ap[st : st + 32, :, q0 + t0 : q0 + 512],
                start=True,
                stop=True,
                perf_mode=DR,
            )

    def emit_exp(qc, hp, ki, ps_tiles, ex_tiles):
        jj = ki - 4 * qc
        t0 = max(0, 128 * jj)
        ps = ps_tiles.pop((hp, ki))
        ex = expp.tile([128, 1024], bf16, name=f"ex{qc}_{hp}_{ki}{sfx}", tag="ex")
        ex_tiles[(hp, ki)] = ex
        if t0:
            nc.scalar.activation(
                ex.rearrange("p (h n) -> p h n", h=2)[:, :, t0:512],
                ps.rearrange("p (h n) -> p h n", h=2)[:, :, t0:512],
                mybir.ActivationFunctionType.Exp,
                scale=float(SCALE_EXP),
            )
        else:
            nc.scalar.activation(
                ex[:], ps[:], mybir.ActivationFunctionType.Exp,
                scale=float(SCALE_EXP),
            )
        if jj >= 0:  # triangular mask on the diagonal 128-block (Pool:
            # SBUF-only op, keeps DVE free; LAG hides the extra hop)
            for h2 in range(2):
                blk = slice(h2 * 512 + t0, h2 * 512 + t0 + 128)
                nc.gpsimd.tensor_mul(ex[:, blk], ex[:, blk], tri_sb[:])

    def emit_ctx(qc, hp, ki, nk, ex_tiles, ctx_ps):
        jj = ki - 4 * qc
        t0 = max(0, 128 * jj)
        q0 = qc * 512
        if ki == 0:
            for h2 in range(2):
                ctx_ps[(hp, h2)] = psctx_pool.tile(
                    [128, 512], f32, name=f"ctx{qc}_{hp}_{h2}{sfx}", tag="ctx"
                )
        ex = ex_tiles.pop((hp, ki))
        for h2 in range(2):
            h = 2 * hp + h2
            nc.tensor.matmul(
                ctx_ps[(hp, h2)][:, t0:512],
                lhsT=v_sb[ki][:, h * 128 : (h + 1) * 128],
                rhs=ex[:, h2 * 512 + t0 : (h2 + 1) * 512],
                start=(ki == 0),
                stop=(ki == nk - 1),
            )

    def emit_normalize(qc, hp, ctx_ps):
        q0 = qc * 512
        for h2 in range(2):
            cp = ctx_ps.pop((hp, h2))
            rec = rpool.tile([64, 512], f32, name=f"rec{qc}_{hp}_{h2}{sfx}", tag="rec")
            nc.vector.reciprocal(rec[:], cp[64:128, :])
            nc.vector.tensor_mul(
                ctxt_sb[hp][h2 * 64 : h2 * 64 + 64, q0 : q0 + 512],
                cp[0:64, :],
                rec[:],
            )

    # ---- global rolling pipeline ----
    # One step per (qc, hp, ki) group, across all chunks: at step s emit
    # scores(s+1) -> exp(s) -> lagged ctx(s-LAG) (+normalize at stream
    # ends, rolling across stream/phase boundaries) -> filler pulled from
    # a deadline-sorted queue under a per-step PE-headroom credit, so PE
    # never outruns the Act exp pace for long.
    order = [0, 1, 2, 3]
    G = []
    phase_start = {}
    for qc in order:
        nk = 4 * qc + 4
        phase_start[qc] = len(G)
        for hp in range(4):
            for ki in range(nk):
                G.append((qc, hp, ki, nk))
    S = len(G)

    # filler queue entries: [deadline, seq, cost_ns, emit_fn, carry_fn]
    queue = []
    seqn = [0]

    def push(deadline, cost, emit, carry=None, carry_cost=0.0):
        queue.append([deadline, seqn[0], cost, emit, carry, carry_cost])
        seqn[0] += 1
        queue.sort()

    # q/k projection groups: u=0 needed at phase start, u=1 by stream 1,
    # u=2 by stream 3 (head->tile mapping per HEADS3).
    for qc in order:
        nk = 4 * qc + 4
        for u, margin in ((0, -4), (1, nk - 4), (2, 3 * nk - 4)):
            if qc == order[0] and u == 0:
                continue  # prologue below
            for w in (0, 1):
                for s in (0, 1):
                    push(
                        phase_start[qc] + margin,
                        428.0,
                        qk_proj_group3(qc, w, u, s),
                    )
    # v tiles: v[k] first consumed by ctx of (qc=k//4, hp=0, ki=k), which
    # pops at phase_start + k + LAG; both halves + copy need a step margin.
    for k in range(NT):
        push(
            max(phase_start[k // 4] + k + LAG - 1, 8),
            853.0,
            v_proj_half(k, 0),
            carry=v_proj_half(k, 1),
            carry_cost=853.0,
        )

    def headroom(w):
        # Act exp pace minus this step's scores+ctx PE cost (ns)
        return 0.4167 * w + 185.0

    # prologue: u=0 q/k groups for the first chunk
    for w in (0, 1):
        for s in (0, 1):
            qk_proj_group3(order[0], w, 0, s)()

    ps_tiles = {}
    ex_tiles = {}
    ctx_ps = {}
    pending = []  # (qc, hp, ki, nk) with exp emitted, ctx not yet emitted
    credit = 0.0
    carry = None
    carry_cost = 0.0

    def pop_ctx():
        cqc, chp, cki, cnk = pending.pop(0)
        emit_ctx(cqc, chp, cki, cnk, ex_tiles, ctx_ps)
        if cki == cnk - 1:
            emit_normalize(cqc, chp, ctx_ps)
            if chp == 3:  # chunk fully normalized: queue its out-proj
                for nt in range(4 * cqc, 4 * cqc + 4):
                    for dh in range(2):
                        push(10 ** 9, 960.0, outproj_half(nt, dh))

    qc0, hp0, ki0, nk0 = G[0]
    emit_scores(qc0, hp0, ki0, ps_tiles)
    for s in range(S):
        qc, hp, ki, nk = G[s]
        if s + 1 < S:
            nqc, nhp, nki, _ = G[s + 1]
            emit_scores(nqc, nhp, nki, ps_tiles)
        emit_exp(qc, hp, ki, ps_tiles, ex_tiles)
        pending.append(G[s])

        credit += headroom(512 - max(0, 128 * (ki - 4 * qc)))
        if carry is not None:  # second half of a split v projection
            carry()
            credit -= carry_cost
            carry = None

        # during the DMA-bound ramp, let pending grow (deep LAG) so PE's
        # wait queue never jams on v tiles; catch up 2 pops/step after
        eff_lag = 10 if s < 16 else LAG
        npop = 0
        while len(pending) > eff_lag and npop < 2:
            pop_ctx()
            npop += 1

        # forced (deadline) and voluntary (credit) filler pulls; a v pull
        # sets `carry` and ends the step so its psum chain stays tight.
        pulled = 0.0
        while queue and carry is None:
            deadline, _, cost, emit, c_fn, c_cost = queue[0]
            if deadline > s and (credit < cost or pulled >= 1100.0):
                break
            queue.pop(0)
            emit()
            credit -= cost
            pulled += cost
            if c_fn is not None:
                carry = c_fn
                carry_cost = c_cost

    # drain: trailing lagged ctx + normalizes, then remaining fillers
    while pending:
        pop_ctx()
    if carry is not None:
        carry()
        carry = None
    drain_mode.append(1)
    while queue:
        _, _, _, emit, c_fn, _ = queue.pop(0)
        emit()
        if c_fn is not None:
            c_fn()


def _build_module(reps=1):
    import concourse.bacc as bacc
    import concourse.mybir as mybir
    import concourse.tile as tile

    f16 = mybir.dt.float16
    f32 = mybir.dt.float32
    bf16 = mybir.dt.bfloat16
    f8 = mybir.dt.float8e4

    nc = bacc.Bacc()
    xt8_d = nc.dram_tensor("xt8", [512, 2 * N_SEQ], f8, kind="ExternalInput")
    wqk8_d = nc.dram_tensor("wqk8", [512, 3072], f8, kind="ExternalInput")
    xtb_d = nc.dram_tensor("xtb", [D_EMB, N_SEQ], bf16, kind="ExternalInput")
    wv_d = nc.dram_tensor("wv", [D_EMB, 512], bf16, kind="ExternalInput")
    wo_d = nc.dram_tensor("wo", [512, D_EMB], bf16, kind="ExternalInput")
    tri_d = nc.dram_tensor("tri", [128, 128], bf16, kind="ExternalInput")
    out_d = nc.dram_tensor("out", [N_SEQ, D_EMB], f16, kind="ExternalOutput")

    with tile.TileContext(nc) as tc:
        with (
            tc.tile_pool(name="persist", bufs=1) as persist,
            tc.tile_pool(name="expp", bufs=12) as expp,
            tc.tile_pool(name="rpool", bufs=4) as rpool,
            tc.tile_pool(name="outp", bufs=3) as outp,
            tc.tile_pool(name="psq", bufs=2, space="PSUM") as psq_pool,
            tc.tile_pool(name="pssc", bufs=2, space="PSUM") as pssc_pool,
            tc.tile_pool(name="psctx", bufs=2, space="PSUM") as psctx_pool,
        ):
            pools = (expp, rpool, outp, psq_pool, pssc_pool, psctx_pool)
            tiles = {
                "xt8": [persist.tile([128, 2 * N_SEQ], f8, name=f"xt8_{i}") for i in range(NPAIR)],
                "wqk8": [
                    persist.tile([128, 3072], f8, name=f"wqk8_{i}")
                    for i in range(NPAIR)
                ],
                "xtb": [persist.tile([128, N_SEQ], bf16, name=f"xtb{k}") for k in range(KT)],
                "wv": [persist.tile([128, 512], bf16, name=f"wv{k}") for k in range(KT)],
                "wo": [persist.tile([128, D_EMB], bf16, name=f"wo{p}") for p in range(4)],
                "qt": [persist.tile([128, 2 * N_SEQ], f8, name=f"qt{u}") for u in range(3)],
                "kt": [persist.tile([128, 2 * N_SEQ], f8, name=f"kt{u}") for u in range(3)],
                # v per n-tile [128, 1024]: head h -> cols h*128:h*128+64 = v_h,
                # cols h*128+64:h*128+128 = 1.0 (softmax denominator ones-trick)
                "v": [persist.tile([128, 1024], bf16, name=f"v{nt}") for nt in range(NT)],
                "ctxt": [persist.tile([128, N_SEQ], bf16, name=f"ctxt{p}") for p in range(4)],
                "tri": persist.tile([128, 128], bf16, name="tri"),
            }
            # ones blocks, once for all reps — on DVE so the Pool engine
            # is free to generate the xtb SWDGE descriptors immediately
            for nt in range(NT):
                ones_view = tiles["v"][nt].rearrange("p (h c) -> p h c", h=8)
                nc.gpsimd.memset(ones_view[:, :, 64:128], 1.0)
            for rep in range(reps):
                _emit_body(
                    nc, tc, mybir, f"_r{rep}" if reps > 1 else "", rep, pools, tiles,
                    xt8_d, wqk8_d, xtb_d, wv_d, wo_d, tri_d, out_d,
                )

    if not nc.is_finalized():
        nc.finalize()
    return nc


def _get_module(reps=1):
    key = f"nc{reps}"
    if key not in _CACHE:
        _CACHE[key] = _build_module(reps)
    return _CACHE[key]


def _pairs(a, ncols):
    # [1024, ncols] -> [512, 2*ncols]: row pair*128+p, col sub*ncols+c
    return (
        a.reshape(NPAIR, 2, 128, ncols).transpose(0, 2, 1, 3).reshape(512, 2 * ncols)
    )


def _f8(a):
    return np.clip(a, -240.0, 240.0).astype(F8E4)


def make_in_maps(x, W_q, W_k, W_v, W_o):
    x = np.asarray(x, np.float32)
    in_maps = []
    for c in range(8):
        b, g = c // 2, c % 2
        gs = slice(g * 512, (g + 1) * 512)
        xT = np.ascontiguousarray(x[b].T)  # [1024, 2048]
        wq = np.asarray(W_q[:, gs], np.float32) * WS
        wk = np.asarray(W_k[:, gs], np.float32) * WS

        # strip layout: per tile u, half s: [head, 32 dims] + pad to 128
        def strip_cols(w):
            cols = []
            for u in range(3):
                for s in (0, 1):
                    blk = np.concatenate(
                        [w[:, 64 * h + 32 * s : 64 * h + 32 * s + 32] for h in HEADS3[u]],
                        axis=1,
                    )
                    if blk.shape[1] < 128:
                        blk = np.concatenate(
                            [blk, np.zeros((1024, 128 - blk.shape[1]), np.float32)],
                            axis=1,
                        )
                    cols.append(blk)
            return np.concatenate(cols, axis=1)  # [1024, 768]

        wqk = np.concatenate([strip_cols(wq), strip_cols(wk)], axis=1)  # [1024, 1536]
        # tri[k_local, q_local] = 1.0 if q_local >= k_local else 0
        tri = np.tril(np.ones((128, 128), np.float32)).T
        in_maps.append(
            {
                "xt8": _f8(_pairs(xT, N_SEQ)),
                "wqk8": _f8(_pairs(wqk, wqk.shape[1])),
                "xtb": xT.astype(BF16),
                "wv": np.ascontiguousarray(W_v[:, gs]).astype(BF16),
                "wo": np.ascontiguousarray(W_o[gs, :]).astype(BF16),
                "tri": np.ascontiguousarray(tri).astype(BF16),
            }
        )
    return in_maps


def kernel(x, W_q, W_k, W_v, W_o, b_o):
    from concourse.bass_utils import run_bass_kernel_spmd

    nc = _get_module()
    in_maps = make_in_maps(x, W_q, W_k, W_v, W_o)
    res = run_bass_kernel_spmd(nc, in_maps, core_ids=list(range(8)))

    out = np.empty((4, N_SEQ, D_EMB), np.float32)
    for b in range(4):
        out[b] = (
            res.results[2 * b]["out"].astype(np.float32)
            + res.results[2 * b + 1]["out"].astype(np.float32)
            + np.asarray(b_o, np.float32)[None, :]
        )
    return out
